# revision 43
# baseline (speedup 1.0000x reference)
"""Trainium2 Bass kernel for the CRF scoring module (nn_CRF_14379550507279).

reference math:
    score0      = transitions[tags[:,0]] + emissions[:,0]            # (B,T)
    trans_steps = transitions[tags[:,:-1], tags[:,1:]] * mask[:,1:]  # (B,S-1)
    emit_steps  = emissions[:,1:,:] * mask[:,1:,None]                # (B,S-1,T)
    total = score0.sum() + trans_steps.sum()*T + emit_steps.sum()

Decomposition (per core, data-parallel over batch):
    total = sum_{b,s,t} emissions[b,s,t] * w[b,s]            (w = mask, w[:,0]=1)
          + 32 * [ sum_{b,s>=1} Tr[prev,next] * mask[b,s]
                   + (1/32) * sum_b rowsumT[tags[b,0]] ]

Per-core layout: 64x2048 (batch, step) grid flattens to (128 partitions, 1024
step-columns); partition p holds batch p//2, steps [(p%2)*1024, +1024).

Implementation (one core):
  * transition sum via a (32,32) pair histogram contracted on the PE:
    bf16 one-hots of (prev+1)*mask and next+1 are built by DVE is_equal at
    the 2x_1p rate using an interleaved (block, tag, gsub) layout -- 4
    step-columns per 128-wide block so every AP has an innermost unit step.
    Each block is one N=128 bf16 matmul psH += A_blk^T B_blk; the 4 chunks
    of a block land on interleaved PSUM rows/cols (t*4+j) and cross-chunk
    products fall on known ignorable positions.  <psH, TrExp> recovers the
    histogram dot with Tr.  A synthetic 257th block (one-hot of tags[b,0]
    against a constant 1/32 row) folds the score0 row-sum lookup in.
  * masked emission sum entirely on the PE: psE += mask8^T ems256 in bf16
    (1 cycle/row at N=256); the fp32->bf16 conversion runs on the otherwise
    idle Scalar engine; diagonal 32-blocks of psE are the per-tag masked
    sums.
  * final: histogram partials -> ones^T @ fin matmul -> (1,1) scalar, plus
    the emask-masked (8,256) emission partial; late-use constants arrive in
    a DMA issued after all emissions tiles; the host sums the partials.

Sharding: batch B=512 split across 8 NeuronCores (64 batches each); host
sums the per-core partials.
"""
import numpy as np
import ml_dtypes

import concourse.bass as bass
import concourse.bacc as bacc
import concourse.mybir as mybir
import concourse.tile as tile
from concourse.bass_utils import run_bass_kernel_spmd

F32 = mybir.dt.float32
F32R = mybir.dt.float32r
BF16 = mybir.dt.bfloat16
I32 = mybir.dt.int32
ALU = mybir.AluOpType
AXL = mybir.AxisListType
ACT = mybir.ActivationFunctionType
BF = ml_dtypes.bfloat16

N_CORES = 8
B, S, T = 512, 2048, 32
BC = B // N_CORES          # 64 batches per core
P = 128                    # SBUF partitions
RPP = BC * S // P          # 1024 step-columns per partition
NBLK = RPP // 4            # 256 4-column histogram blocks
CHB = 32                   # blocks per one-hot build chunk
NCH = NBLK // CHB          # 8 chunks
G = 128                    # emission step-columns per DMA tile
NT = RPP // G              # 8 emission tiles
EG = 8                     # emission columns per matmul (N = EG*T = 256)

_cached = {}


def _build(repeat=1):
    nc = bacc.Bacc("TRN2", target_bir_lowering=False, debug=False)

    ems = nc.dram_tensor("ems", [P, RPP, T], F32, kind="ExternalInput")
    msk = nc.dram_tensor("msk", [P, RPP], F32, kind="ExternalInput")
    tgn = nc.dram_tensor("tgn", [P, RPP], I32, kind="ExternalInput")
    # packed constants:
    #   cfe = [cst(2)] fp32 (needed early)
    #   cfl = [trexp(128) | emask(256) | ones(1)] fp32 (needed only in the
    #         finals -- DMA'd AFTER all emissions tiles so the last emissions
    #         byte lands earlier)
    #   cfb = [psy(4) | iota(128) | bsyn(128) | pv0(1)] bf16
    # (pv0 = tags[p-1, RPP-1], the cross-partition shifted first-column
    # previous tag, materialized host-side -- pure input reindexing)
    cfe = nc.dram_tensor("cfe", [P, 2], F32, kind="ExternalInput")
    cfl = nc.dram_tensor("cfl", [P, 128 + EG * T + 1], F32,
                         kind="ExternalInput")
    cfb = nc.dram_tensor("cfb", [P, 4 + 128 + 128 + 1], BF16,
                         kind="ExternalInput")
    # two partial outputs summed on the host: histogram scalar (reduced on
    # device, off the critical path) and the masked-emission partial (rows
    # 0..7, diagonal-extracted via emask; only 8 DMA descriptors)
    outh = nc.dram_tensor("outh", [1, 1], F32, kind="ExternalOutput")
    oute = nc.dram_tensor("oute", [EG, EG * T], F32, kind="ExternalOutput")

    with tile.TileContext(nc) as tc:
        with (
            tc.tile_pool(name="pers", bufs=2) as pers,
            tc.tile_pool(name="ohpA", bufs=2) as ohpA,
            tc.tile_pool(name="ohpB", bufs=2) as ohpB,
            tc.tile_pool(name="epool", bufs=3) as epool,
            tc.tile_pool(name="psum", bufs=2, space="PSUM") as psump,
        ):
          for _rep in range(repeat):
            # ---------- prefetch emissions tile 0 ahead of everything ------
            # (the DMA queue drains in program order; the critical path ends
            # with the LAST emissions byte, so small inputs go after tile 0)
            H = G * T // 2
            et0 = epool.tile([P, G * T], F32, tag="et", bufs=4)
            for h in range(2):
                nc.sync.dma_start(
                    et0[:, h * H:(h + 1) * H]
                        .rearrange("p (g t) -> p g t", t=T),
                    ems[:, h * G // 2:(h + 1) * G // 2, :])

            # ---------- constant + input loads ----------
            cft = pers.tile([P, 2], F32, tag="cft")
            nc.sync.dma_start(cft[:], cfe[:])
            cbt = pers.tile([P, 4 + 128 + 128 + 1], BF16, tag="cbt")
            nc.sync.dma_start(cbt[:], cfb[:])
            m = pers.tile([P, RPP], F32, tag="m")
            nc.sync.dma_start(m[:], msk[:])
            tg = pers.tile([P, RPP], I32, tag="tg")
            nc.sync.dma_start(tg[:], tgn[:])

            # ---------- index / weight prep (DVE) ----------
            # mc0 = mask[:,0] * valid0  (transition weight for column 0)
            mc0 = pers.tile([P, 1], F32, tag="mc0")
            nc.vector.tensor_tensor(mc0[:], m[:, 0:1], cft[:, 0:1], ALU.mult)
            # pmt = (prev+1)*m_trans  (0 never matches iota values 1..32)
            pmt = pers.tile([P, RPP], BF16, tag="pmt")
            nc.vector.scalar_tensor_tensor(
                out=pmt[:, 1:RPP], in0=tg[:, 0:RPP - 1], scalar=1.0,
                in1=m[:, 1:RPP], op0=ALU.add, op1=ALU.mult)
            nc.vector.scalar_tensor_tensor(
                out=pmt[:, 0:1], in0=cbt[:, 260:261], scalar=1.0, in1=mc0[:],
                op0=ALU.add, op1=ALU.mult)
            # emission weight for column 0: mask*valid0 + (1-valid0)
            nc.vector.tensor_tensor(m[:, 0:1], mc0[:], cft[:, 1:2], ALU.add)
            # nxf = next+1
            nxf = pers.tile([P, RPP], BF16, tag="nxf")
            nc.vector.tensor_scalar_add(nxf[:], tg[:], 1.0)
            # bf16 emission weights (after the column-0 fix above)
            mb = pers.tile([P, RPP], BF16, tag="mb")
            nc.vector.tensor_copy(mb[:], m[:])
            # synthetic block A: one-hot of tags[b,0]+1 on even partitions
            asyn = pers.tile([P, 128], BF16, tag="asyn")
            nc.vector.tensor_tensor(
                asyn[:].rearrange("p (t j) -> p t j", j=4),
                cbt[:, 0:4].unsqueeze(1).broadcast_to((P, 32, 4)),
                cbt[:, 4:132].rearrange("p (t j) -> p t j", j=4),
                ALU.is_equal)

            # ---------- PSUM accumulators ----------
            psH = psump.tile([P, 128], F32, tag="psH")
            psE = psump.tile([P, EG * T], F32, tag="psE")

            # synthetic histogram block: rowsum lookup, pre-scaled by 1/32
            nc.tensor.matmul(psH[:, :], asyn[:], cbt[:, 132:260],
                             start=True, stop=False)

            # ---------- main loop: one-hot chunks + hist & emis matmuls ----
            # NT tile-iterations; a one-hot chunk covers CPT = NBLK//NT
            # blocks per iteration and is built every NCH/NT iterations
            CPT = NBLK // NT
            At = Bt = None
            for c in range(NT):
                ch, sub = divmod(c * CPT, CHB)
                if sub == 0:
                    At = ohpA.tile([P, CHB * 128], BF16, tag="At")
                    nc.vector.tensor_tensor(
                        At[:].rearrange("p (b t j) -> p b t j", t=32, j=4),
                        pmt[:, ch * CHB * 4:(ch + 1) * CHB * 4]
                            .rearrange("p (b j) -> p b j", j=4)
                            .unsqueeze(2).broadcast_to((P, CHB, 32, 4)),
                        cbt[:, 4:132].rearrange("p (t j) -> p t j", j=4)
                            .unsqueeze(1).broadcast_to((P, CHB, 32, 4)),
                        ALU.is_equal)
                    Bt = ohpB.tile([P, CHB * 128], BF16, tag="Bt")
                    nc.vector.tensor_tensor(
                        Bt[:].rearrange("p (b t j) -> p b t j", t=32, j=4),
                        nxf[:, ch * CHB * 4:(ch + 1) * CHB * 4]
                            .rearrange("p (b j) -> p b j", j=4)
                            .unsqueeze(2).broadcast_to((P, CHB, 32, 4)),
                        cbt[:, 4:132].rearrange("p (t j) -> p t j", j=4)
                            .unsqueeze(1).broadcast_to((P, CHB, 32, 4)),
                        ALU.is_equal)
                for b in range(sub, sub + CPT):
                    nc.tensor.matmul(
                        psH[:, :],
                        At[:, b * 128:(b + 1) * 128],
                        Bt[:, b * 128:(b + 1) * 128],
                        start=False,
                        stop=(c == NT - 1 and b == sub + CPT - 1))
                # emissions tile c: DMA, Scalar-engine bf16 cast, then
                # mask^T @ ems matmuls (half-tile granularity for pipelining)
                last = c == NT - 1
                if c == 0:
                    et = et0
                else:
                    et = epool.tile([P, G * T], F32, tag="et", bufs=4)
                    # last tile: quarter-granularity DMAs so the tail cast
                    # starts on the last quarter, not the last half
                    nd = 4 if last else 2
                    HD = G * T // nd
                    for h in range(nd):
                        nc.sync.dma_start(
                            et[:, h * HD:(h + 1) * HD]
                                .rearrange("p (g t) -> p g t", t=T),
                            ems[:, c * G + h * G // nd:
                                c * G + (h + 1) * G // nd, :])
                etb = epool.tile([P, G * T], BF16, tag="etb", bufs=3)
                if last:
                    # first half on the Scalar engine, the last two quarters
                    # on the (by now idle) DVE so the tail doesn't serialize
                    # behind one engine
                    nc.scalar.activation(etb[:, 0:H], et[:, 0:H], ACT.Copy)
                    Q = H // 2
                    nc.vector.tensor_copy(etb[:, H:H + Q], et[:, H:H + Q])
                    nc.vector.tensor_copy(etb[:, H + Q:], et[:, H + Q:])
                else:
                    for h in range(2):
                        nc.scalar.activation(etb[:, h * H:(h + 1) * H],
                                             et[:, h * H:(h + 1) * H],
                                             ACT.Copy)
                for k in range(G // EG):
                    g0 = c * G + k * EG
                    nc.tensor.matmul(
                        psE[0:EG, :],
                        mb[:, g0:g0 + EG],
                        etb[:, k * EG * T:(k + 1) * EG * T],
                        start=(c == 0 and k == 0),
                        stop=(c == NT - 1 and k == G // EG - 1))

            # ---------- finals (host sums the partials) ----------
            # late constants land right after the last emissions byte
            cflt = pers.tile([P, 128 + EG * T + 1], F32, tag="cflt")
            nc.sync.dma_start(cflt[:], cfl[:])
            csb = pers.tile([P, 128], F32, tag="csb")
            nc.vector.tensor_tensor(csb[:], psH[:, :], cflt[:, 0:128],
                                    ALU.mult)
            fin = pers.tile([P, 1], F32, tag="fin")
            nc.vector.tensor_reduce(fin[:], csb[:], axis=AXL.X, op=ALU.add)
            psF = psump.tile([1, 1], F32, tag="psF")
            nc.tensor.matmul(psF[:], cflt[:, 384:385], fin[:],
                             start=True, stop=True)
            osb = pers.tile([1, 1], F32, tag="osb")
            nc.vector.tensor_copy(osb[:], psF[:])
            # emission partial: emask zeroes the off-diagonal garbage
            esb = pers.tile([P, EG * T], F32, tag="esb")
            nc.vector.tensor_tensor(esb[0:EG, :], psE[0:EG, :],
                                    cflt[0:EG, 128:128 + EG * T], ALU.mult)
            nc.sync.dma_start(oute[:], esb[0:EG, :])
            nc.sync.dma_start(outh[:], osb[:])
    nc.compile()
    return nc


def _consts():
    parity = (np.arange(P) % 2).astype(np.float32)     # 0 even, 1 odd
    cst = np.zeros((P, 2), np.float32)
    cst[:, 0] = parity                                 # valid0
    cst[:, 1] = 1.0 - parity                           # 1 - valid0
    # iota+1 in interleaved (t, j) layout: value t+1 at index t*4+j
    iot = np.repeat(np.arange(1, 33, dtype=np.float32), 4)
    iot = np.ascontiguousarray(np.broadcast_to(iot.astype(BF), (P, 128)))
    # synthetic moving block: 1/32 at j==0, else 0
    bsy = np.zeros(128, np.float32)
    bsy[0::4] = 1.0 / 32.0
    bsy = np.ascontiguousarray(np.broadcast_to(bsy.astype(BF), (P, 128)))
    # emissions diagonal-extract mask: row j keeps columns [32j, 32j+32)
    emk = np.zeros((P, EG * T), np.float32)
    for j in range(EG):
        emk[j, T * j:T * (j + 1)] = 1.0
    return cst, iot, bsy, emk


def _trexp(transitions):
    # TrExp[(t,j),(t',j')] = 32*Tr[t,t'] if j==j' else 0 ; rows = partitions.
    # The num_tags factor is folded in here (the synthetic rowsum block uses
    # a 1/32 moving column, so it comes out exactly unscaled).
    t4 = np.zeros((32, 4, 32, 4), np.float32)
    for j in range(4):
        t4[:, j, :, j] = 32.0 * transitions
    return np.ascontiguousarray(t4.reshape(128, 128))


def _in_maps(emissions, tags, mask, transitions):
    cst, iot, bsy, emk = _consts()
    trx = _trexp(np.asarray(transitions, np.float32))
    ones = np.ones((P, 1), np.float32)
    cfe = np.ascontiguousarray(cst, np.float32)
    cfl = np.ascontiguousarray(
        np.concatenate([trx, emk, ones], axis=1), np.float32)
    parity = np.arange(P) % 2
    maps = []
    for c in range(N_CORES):
        sl = slice(c * BC, (c + 1) * BC)
        tg0 = np.repeat(tags[sl, 0], 2).astype(np.float32)   # (P,)
        psy = np.zeros((P, 4), np.float32)
        psy[:, 0] = np.where(parity == 0, tg0 + 1.0, 0.0)
        # pv0[p] = tags-grid[p-1, RPP-1]: previous tag feeding column 0
        tgr = tags[sl].reshape(P, RPP)
        pv0 = np.zeros((P, 1), np.float32)
        pv0[1:, 0] = tgr[:-1, RPP - 1]
        cfb = np.ascontiguousarray(np.concatenate(
            [psy.astype(BF), iot, bsy, pv0.astype(BF)], axis=1))
        maps.append(dict(
            ems=np.ascontiguousarray(emissions[sl]).reshape(P, RPP, T),
            msk=np.ascontiguousarray(mask[sl]).reshape(P, RPP),
            tgn=np.ascontiguousarray(tags[sl]).reshape(P, RPP),
            cfe=cfe,
            cfl=cfl,
            cfb=cfb,
        ))
    return maps


def kernel(emissions, tags, mask, transitions):
    emissions = np.asarray(emissions, np.float32)
    tags = np.asarray(tags, np.int32)
    mask = np.asarray(mask, np.float32)
    transitions = np.asarray(transitions, np.float32)

    if "nc" not in _cached:
        _cached["nc"] = _build()
    nc = _cached["nc"]
    maps = _in_maps(emissions, tags, mask, transitions)
    res = run_bass_kernel_spmd(nc, maps, list(range(N_CORES)))
    total = np.float64(0.0)
    for c in range(N_CORES):
        total += np.float64(res.results[c]["outh"][0, 0])
        total += np.float64(res.results[c]["oute"]).sum()
    return np.float32(total)
